# revision 1
# baseline (speedup 1.0000x reference)
"""Trainium2 Bass kernel for a dense transformer block (nn_Block_58377195487260).

Reference (per batch element, fp32):
    h   = LN1(x)*g1 + b1ln
    q,k,v = h@wq, h@wk, h@wv
    s   = q@k^T / sqrt(dk);  a = softmax(s);  y = (a@v)@wo
    x2  = h + y
    mlp = gelu(LN2(x2)*g2 + b2ln @ w1 + b1) @ w2 + b2
    out = x2 + mlp

Sharding: data-parallel over batch. B=8 == 8 NeuronCores; core i computes
batch element i end-to-end (no collectives).

On-chip dataflow is kept in feature-major ("transposed") layout [d, s] so
every matmul consumes operands in natural layout and every bias/gain lands
on the partition axis:
    hT (bf16)   <- PE-transpose of LN1(x)            [d, s]
    qT, kT      <- wq/wk-stationary matmuls over hT  [dk, s]
    V           <- hT-stationary matmul with wv      [s, dv]
    ST          <- kT.T @ qT                         [sk, sq]   (scores^T)
    ET          <- exp(ST/sqrt(dk))   (no max-subtract: |s| < ~6 is safe)
    sums        <- ones.T @ ET        (partition reduction on PE)
    UT          <- V.T @ ET           (accumulate over sk)  [dv, sq]
    yTs         <- UT * broadcast(1/sums)
    x2T         <- hT + wo.T @ yTs                   [d, s]  (spilled to DRAM)
    LN2         <- partition-dim mean/var via ones-matmuls
    GT          <- gelu(w1.T @ h2T + b1)             [h, s]
    outT        <- x2T + w2.T @ GT + b2              [d, s]
    out         <- PE-transpose back to [s, d]

LN1 and QKV are fused per 512-column s-block so the tensor engine gets
dense matmul work early (HAM warm-up) while later s-chunks are still in
layernorm. Matmuls run in bf16 with fp32 PSUM accumulation; LN statistics,
softmax normalization and residual adds stay fp32.
"""

import numpy as np
import ml_dtypes
from contextlib import ExitStack

P = 128
B, S, D, H = 8, 2048, 1024, 4096
DC = D // P          # 8  d-chunks
HC = H // P          # 32 h-chunks
SC = S // P          # 16 s-chunks
QB = 512             # attention sq-block
NQB = S // QB        # 4
MB = 512             # mlp/ln2 s-block
NMB = S // MB        # 4
EPS = 1e-5
SM_SCALE = 1.0 / 32.0   # 1/sqrt(1024)

N_CORES = 8


def build(nc, bass, mybir, tile):
    f32 = mybir.dt.float32
    bf16 = mybir.dt.bfloat16

    x_in = nc.declare_dram_parameter("x", [S, D], f32, isOutput=False)
    # qkv weights arrive pre-tiled: [out_chunk, d_chunk, d_in, out_in] so the
    # per-out-chunk slice is one contiguous 256 KB DMA
    wq_in = nc.declare_dram_parameter("wq", [DC, DC, P, P], bf16, isOutput=False)
    wk_in = nc.declare_dram_parameter("wk", [DC, DC, P, P], bf16, isOutput=False)
    wv_in = nc.declare_dram_parameter("wv", [D, D], bf16, isOutput=False)
    wo_in = nc.declare_dram_parameter("wo", [D, D], bf16, isOutput=False)
    # w1 arrives pre-scaled by ln2_g; w1gs = -sum_d w1[d,h]*g2[d]; b1 folds
    # in w1.T @ ln2_b (LN2 is folded into the GT matmul algebraically)
    w1_in = nc.declare_dram_parameter("w1", [HC, DC, P, P], bf16, isOutput=False)
    w1gs_in = nc.declare_dram_parameter("w1gs", [H], f32, isOutput=False)
    w2_in = nc.declare_dram_parameter("w2", [H, D], bf16, isOutput=False)
    ln1g_in = nc.declare_dram_parameter("ln1_g", [D], f32, isOutput=False)
    ln1b_in = nc.declare_dram_parameter("ln1_b", [D], f32, isOutput=False)
    ln2g_in = nc.declare_dram_parameter("ln2_g", [D], f32, isOutput=False)
    ln2b_in = nc.declare_dram_parameter("ln2_b", [D], f32, isOutput=False)
    b1_in = nc.declare_dram_parameter("b1", [H], f32, isOutput=False)
    b2_in = nc.declare_dram_parameter("b2", [D], f32, isOutput=False)
    out_dram = nc.declare_dram_parameter("out", [S, D], f32, isOutput=True)

    from concourse.masks import make_identity

    with tile.TileContext(nc) as tc, ExitStack() as top:
        const = top.enter_context(tc.tile_pool(name="const", bufs=1))
        dram = top.enter_context(tc.tile_pool(name="dram", bufs=1, space="DRAM"))

        ident = const.tile([P, P], f32)
        make_identity(nc, ident)
        eps_p = const.tile([P, 1], f32)
        nc.vector.memset(eps_p, EPS)
        eps_1 = const.tile([1, 1], f32)
        nc.vector.memset(eps_1, EPS)
        ones_bf = const.tile([P, 1], bf16)
        nc.vector.memset(ones_bf, 1.0)
        ones_row = const.tile([1, P], f32)
        nc.vector.memset(ones_row, 1.0)

        # per-partition views of gains/biases: [P, nchunk], column c = chunk c
        ln1g = const.tile([P, DC], f32)
        ln1b = const.tile([P, DC], f32)
        ln2g = const.tile([P, DC], f32)
        ln2b = const.tile([P, DC], f32)
        b1c = const.tile([P, HC], f32)
        b2c = const.tile([P, DC], f32)
        w1gs = const.tile([P, HC], f32)
        for dst, src in ((ln1g, ln1g_in), (ln1b, ln1b_in),
                         (ln2g, ln2g_in), (ln2b, ln2b_in),
                         (b1c, b1_in), (b2c, b2_in), (w1gs, w1gs_in)):
            nc.sync.dma_start(out=dst, in_=src.rearrange("(c p) -> p c", p=P))

        x2T_dram = dram.tile([P, DC, S], f32)    # x2 in [d, s] layout
        # bf16 copy of x2T's first MLP block, filled during Phase 3 straight
        # from the x2w tiles so Phase 4/5's first GT matmuls don't wait on
        # the DRAM round-trip + pool drain at the phase transition
        bt0 = const.tile([P, DC, MB], bf16)

        import os
        for _rep in range(int(os.environ.get("BENCH_REPS", "1"))):
            _build_body(nc, tc, mybir, locals())

    nc.finalize()
    return nc


def _build_body(nc, tc, mybir, env):
    f32 = mybir.dt.float32
    bf16 = mybir.dt.bfloat16
    AF = mybir.ActivationFunctionType
    ALU = mybir.AluOpType
    (x_in, wq_in, wk_in, wv_in, wo_in, w1_in, w2_in, out_dram, x2T_dram,
     ident, eps_p, eps_1, ones_bf, ones_row,
     ln1g, ln1b, ln2g, ln2b, b1c, b2c, w1gs) = (
        env["x_in"], env["wq_in"], env["wk_in"], env["wv_in"], env["wo_in"],
        env["w1_in"], env["w2_in"], env["out_dram"], env["x2T_dram"],
        env["ident"], env["eps_p"], env["eps_1"], env["ones_bf"],
        env["ones_row"], env["ln1g"], env["ln1b"], env["ln2g"], env["ln2b"],
        env["b1c"], env["b2c"], env["w1gs"])
    bt0 = env["bt0"]

    with ExitStack() as ph03:
        act = ph03.enter_context(tc.tile_pool(name="act", bufs=1))
        hT = act.tile([P, DC, S], bf16)          # 4 MB, [d, s]
        qT = act.tile([P, DC, S], bf16)          # 4 MB, [dk, s]
        kT = act.tile([P, DC, S], bf16)          # 4 MB, [dk, s]
        V = act.tile([P, SC, D], bf16)           # 4 MB, [s, dv]

        # ---- Phase 0-2 fused per 512-col s-block: LN1 + transpose + QKV ----
        with ExitStack() as ph:
            xp = ph.enter_context(tc.tile_pool(name="xp", bufs=3))
            hp = ph.enter_context(tc.tile_pool(name="hp", bufs=3))
            st = ph.enter_context(tc.tile_pool(name="st", bufs=4))
            wsp = ph.enter_context(tc.tile_pool(name="wsp", bufs=6))
            wvp = ph.enter_context(tc.tile_pool(name="wvp", bufs=1))
            tps = ph.enter_context(
                tc.tile_pool(name="tps", bufs=4, space="PSUM"))
            mps = ph.enter_context(
                tc.tile_pool(name="mps", bufs=4, space="PSUM"))
            wv_sb = wvp.tile([P, DC, D], bf16)
            wv_view = wv_in.rearrange("(c p) n -> p c n", p=P)
            for g in range(4):
                nc.sync.dma_start(out=wv_sb[:, g * 2:(g + 1) * 2, :],
                                  in_=wv_view[:, g * 2:(g + 1) * 2, :])
            for sb in range(4):
                for sc in range(4 * sb, 4 * sb + 4):
                    x_t = xp.tile([P, D], f32, tag="x")
                    nc.sync.dma_start(out=x_t, in_=x_in[sc * P:(sc + 1) * P, :])
                    stats = st.tile([P, 2, 6], f32, tag="stats")
                    nc.vector.bn_stats(out=stats[:, 0, :], in_=x_t[:, 0:512])
                    nc.vector.bn_stats(out=stats[:, 1, :], in_=x_t[:, 512:1024])
                    mv = st.tile([P, 2], f32, tag="mv")
                    nc.vector.bn_aggr(out=mv, in_=stats)
                    std = st.tile([P, 1], f32, tag="std")
                    nc.scalar.activation(out=std, in_=mv[:, 1:2], func=AF.Sqrt,
                                         bias=eps_p)
                    rstd = st.tile([P, 1], f32, tag="rstd")
                    nc.vector.reciprocal(out=rstd, in_=std)
                    h_t = hp.tile([P, D], f32, tag="h")
                    nc.vector.tensor_scalar(out=h_t, in0=x_t,
                                            scalar1=mv[:, 0:1], scalar2=rstd,
                                            op0=ALU.subtract, op1=ALU.mult)
                    for dc in range(DC):
                        tp = tps.tile([P, P], f32, tag="tp")
                        nc.tensor.transpose(tp, h_t[:, dc * P:(dc + 1) * P],
                                            ident)
                        nc.vector.tensor_scalar(
                            out=hT[:, dc, sc * P:(sc + 1) * P], in0=tp,
                            scalar1=ln1g[:, dc:dc + 1],
                            scalar2=ln1b[:, dc:dc + 1],
                            op0=ALU.mult, op1=ALU.add)
                # qT / kT for this s-block (weights streamed per out-chunk)
                for dst, w_in in ((qT, wq_in), (kT, wk_in)):
                    for jc in range(DC):
                        wt = wsp.tile([P, DC, P], bf16, tag="wt")
                        nc.sync.dma_start(
                            out=wt, in_=w_in[jc].rearrange("c p n -> p c n"))
                        ps = mps.tile([P, 512], f32, tag="ps")
                        for dc in range(DC):
                            nc.tensor.matmul(
                                ps, wt[:, dc, :],
                                hT[:, dc, sb * 512:(sb + 1) * 512],
                                start=(dc == 0), stop=(dc == DC - 1))
                        o = dst[:, jc, sb * 512:(sb + 1) * 512]
                        if jc % 2 == 0:
                            nc.vector.tensor_copy(o, ps)
                        else:
                            nc.scalar.copy(o, ps)
                # V rows for this s-block
                for skc in range(4 * sb, 4 * sb + 4):
                    for db in range(2):
                        ps = mps.tile([P, 512], f32, tag="ps")
                        for dc in range(DC):
                            nc.tensor.matmul(
                                ps, hT[:, dc, skc * P:(skc + 1) * P],
                                wv_sb[:, dc, db * 512:(db + 1) * 512],
                                start=(dc == 0), stop=(dc == DC - 1))
                        o = V[:, skc, db * 512:(db + 1) * 512]
                        if (skc + db) % 2 == 0:
                            nc.vector.tensor_copy(o, ps)
                        else:
                            nc.scalar.copy(o, ps)

        # ------------- Phase 3: attention + wo + residual -------------
        with ExitStack() as ph:
            wop = ph.enter_context(tc.tile_pool(name="wop", bufs=1))
            etp = ph.enter_context(tc.tile_pool(name="etp", bufs=1))
            ytp = ph.enter_context(tc.tile_pool(name="ytp", bufs=1))
            rbp = ph.enter_context(tc.tile_pool(name="rbp", bufs=2))
            x2p = ph.enter_context(tc.tile_pool(name="x2p", bufs=3))
            rcp = ph.enter_context(tc.tile_pool(name="rcp", bufs=2))
            sps = ph.enter_context(
                tc.tile_pool(name="sps", bufs=2, space="PSUM"))
            ups = ph.enter_context(
                tc.tile_pool(name="ups", bufs=2, space="PSUM"))
            smps = ph.enter_context(
                tc.tile_pool(name="smps", bufs=2, space="PSUM"))

            wo_sb = wop.tile([P, DC, D], bf16)
            wo_view = wo_in.rearrange("(c p) n -> p c n", p=P)
            for g in range(4):
                nc.sync.dma_start(out=wo_sb[:, g * 2:(g + 1) * 2, :],
                                  in_=wo_view[:, g * 2:(g + 1) * 2, :])

            for qb in range(NQB):
                q0 = qb * QB
                ET = etp.tile([P, SC, QB], bf16, tag="ET")
                for skc in range(SC):
                    ps = sps.tile([P, QB], f32, tag="st")
                    for jc in range(DC):
                        nc.tensor.matmul(
                            ps, kT[:, jc, skc * P:(skc + 1) * P],
                            qT[:, jc, q0:q0 + QB],
                            start=(jc == 0), stop=(jc == DC - 1))
                    nc.scalar.activation(out=ET[:, skc, :], in_=ps,
                                         func=AF.Exp, scale=SM_SCALE)
                # partition-sum of ET via ones-matmuls
                sum_ps = smps.tile([1, QB], f32, tag="sm")
                for skc in range(SC):
                    nc.tensor.matmul(sum_ps, ones_bf, ET[:, skc, :],
                                     start=(skc == 0), stop=(skc == SC - 1))
                recip = rcp.tile([1, QB], f32, tag="recip")
                nc.vector.reciprocal(out=recip, in_=sum_ps)
                # broadcast recip over partitions via K=1 fp32 matmul
                rb_ps = smps.tile([P, QB], f32, tag="sm")
                nc.tensor.matmul(rb_ps, ones_row, recip, start=True, stop=True)
                Rb = rbp.tile([P, QB], f32, tag="Rb")
                nc.vector.tensor_copy(Rb, rb_ps)
                # UT = V.T @ ET, scaled by Rb
                yTs = ytp.tile([P, DC, QB], bf16, tag="yTs")
                for dvc in range(DC):
                    ps = ups.tile([P, QB], f32, tag="ps")
                    for skc in range(SC):
                        nc.tensor.matmul(
                            ps, V[:, skc, dvc * P:(dvc + 1) * P],
                            ET[:, skc, :],
                            start=(skc == 0), stop=(skc == SC - 1))
                    nc.vector.tensor_tensor(out=yTs[:, dvc, :], in0=ps,
                                            in1=Rb, op=ALU.mult)
                # x2T = hT + wo.T @ yTs  -> DRAM
                for dc in range(DC):
                    ps = ups.tile([P, QB], f32, tag="ps")
                    for dvc in range(DC):
                        nc.tensor.matmul(
                            ps, wo_sb[:, dvc, dc * P:(dc + 1) * P],
                            yTs[:, dvc, :],
                            start=(dvc == 0), stop=(dvc == DC - 1))
                    x2w = x2p.tile([P, QB], f32, tag="x2w")
                    nc.vector.tensor_tensor(out=x2w, in0=ps,
                                            in1=hT[:, dc, q0:q0 + QB],
                                            op=ALU.add)
                    nc.sync.dma_start(out=x2T_dram[:, dc, q0:q0 + QB],
                                      in_=x2w)
                    if qb == 0:
                        nc.vector.tensor_copy(bt0[:, dc, :], x2w)

    # ------------- Phase 4/5: LN2 + MLP + out -------------
    with ExitStack() as ph:
        w2p = ph.enter_context(tc.tile_pool(name="w2p", bufs=1))
        w1p = ph.enter_context(tc.tile_pool(name="w1p", bufs=6))
        x2b = ph.enter_context(tc.tile_pool(name="x2b", bufs=2))
        bfp = ph.enter_context(tc.tile_pool(name="bfp", bufs=16))
        sqp = ph.enter_context(tc.tile_pool(name="sqp", bufs=8))
        gtt = ph.enter_context(tc.tile_pool(name="gtt", bufs=3))
        stp = ph.enter_context(tc.tile_pool(name="stp", bufs=4))
        bcp = ph.enter_context(tc.tile_pool(name="bcp", bufs=4))
        gtp = ph.enter_context(tc.tile_pool(name="gtp", bufs=1))
        otp = ph.enter_context(tc.tile_pool(name="otp", bufs=3))
        sgp = ph.enter_context(tc.tile_pool(name="sgp", bufs=6))
        gps = ph.enter_context(tc.tile_pool(name="gps", bufs=2, space="PSUM"))
        mps2 = ph.enter_context(
            tc.tile_pool(name="mps2", bufs=2, space="PSUM"))
        lps = ph.enter_context(tc.tile_pool(name="lps", bufs=2, space="PSUM"))
        tps2 = ph.enter_context(
            tc.tile_pool(name="tps2", bufs=2, space="PSUM"))

        w2_sb = w2p.tile([P, HC, D], bf16)
        w2_view = w2_in.rearrange("(c p) n -> p c n", p=P)
        for g in range(8):
            nc.sync.dma_start(out=w2_sb[:, g * 4:(g + 1) * 4, :],
                              in_=w2_view[:, g * 4:(g + 1) * 4, :])

        for mb in range(NMB):
            s0 = mb * MB
            x2Tb = x2b.tile([P, DC, MB], f32, tag="x2Tb")
            for dc in range(DC):
                nc.sync.dma_start(out=x2Tb[:, dc, :],
                                  in_=x2T_dram[:, dc, s0:s0 + MB])
            # LN2 stats: partition sums of x2 and x2^2 (bf16 matmuls)
            bts = []
            for dc in range(DC):
                if mb == 0:
                    bt = bt0[:, dc, :]
                else:
                    bt = bfp.tile([P, MB], bf16, tag="bt")
                    nc.vector.tensor_copy(bt, x2Tb[:, dc, :])
                sq = sqp.tile([P, MB], bf16, tag="sq")
                nc.scalar.activation(out=sq, in_=bt, func=AF.Square)
                bts.append((bt, sq))
            sum_ps = lps.tile([1, MB], f32, tag="lp")
            for dc in range(DC):
                nc.tensor.matmul(sum_ps, ones_bf, bts[dc][0],
                                 start=(dc == 0), stop=(dc == DC - 1))
            sq_ps = lps.tile([1, MB], f32, tag="lp")
            for dc in range(DC):
                nc.tensor.matmul(sq_ps, ones_bf, bts[dc][1],
                                 start=(dc == 0), stop=(dc == DC - 1))
            mu = stp.tile([1, MB], f32, tag="stat")
            nc.scalar.activation(out=mu, in_=sum_ps, func=AF.Copy,
                                 scale=1.0 / D)
            msq = stp.tile([1, MB], f32, tag="stat")
            nc.scalar.activation(out=msq, in_=sq_ps, func=AF.Copy,
                                 scale=1.0 / D)
            var = stp.tile([1, MB], f32, tag="stat")
            nc.vector.tensor_tensor(out=var, in0=mu, in1=mu, op=ALU.mult)
            nc.vector.tensor_tensor(out=var, in0=msq, in1=var,
                                    op=ALU.subtract)
            stdv = stp.tile([1, MB], f32, tag="stat")
            nc.scalar.activation(out=stdv, in_=var, func=AF.Sqrt,
                                 bias=eps_1)
            rstd = stp.tile([1, MB], f32, tag="stat")
            nc.vector.reciprocal(out=rstd, in_=stdv)
            avec = stp.tile([1, MB], f32, tag="stat")
            nc.vector.tensor_tensor(out=avec, in0=mu, in1=rstd, op=ALU.mult)
            rstd_bc = bcp.tile([P, MB], f32, tag="bc")
            a_bc = bcp.tile([P, MB], f32, tag="bc")
            for vec, bc in ((rstd, rstd_bc), (avec, a_bc)):
                bc_ps = lps.tile([P, MB], f32, tag="lp")
                nc.tensor.matmul(bc_ps, ones_row, vec, start=True,
                                 stop=True)
                nc.vector.tensor_copy(bc, bc_ps)
            # GT = gelu(rstd*(w1g2.T @ x2T) - (mu*rstd)*w1gs + b1_eff):
            # LN2 is folded into the matmul, so PE needn't wait for stats
            GTb = gtp.tile([P, HC, MB], bf16, tag="GTb")
            for hc in range(HC):
                w1t = w1p.tile([P, DC, P], bf16, tag="w1t")
                nc.sync.dma_start(
                    out=w1t, in_=w1_in[hc].rearrange("c p n -> p c n"))
                ps = gps.tile([P, MB], f32, tag="gt")
                for dc in range(DC):
                    nc.tensor.matmul(
                        ps, w1t[:, dc, :], bts[dc][0],
                        start=(dc == 0), stop=(dc == DC - 1))
                t1 = gtt.tile([P, MB], f32, tag="t1")
                nc.vector.tensor_tensor(out=t1, in0=ps, in1=rstd_bc,
                                        op=ALU.mult)
                t2 = gtt.tile([P, MB], f32, tag="t1")
                nc.vector.scalar_tensor_tensor(
                    out=t2, in0=a_bc, scalar=w1gs[:, hc:hc + 1], in1=t1,
                    op0=ALU.mult, op1=ALU.add)
                nc.scalar.activation(out=GTb[:, hc, :], in_=t2,
                                     func=AF.Gelu,
                                     bias=b1c[:, hc:hc + 1])
            # outT = x2T + w2.T @ GT + b2; transpose to natural layout
            for dc in range(DC):
                ps = mps2.tile([P, MB], f32, tag="mo")
                for hc in range(HC):
                    nc.tensor.matmul(
                        ps, w2_sb[:, hc, dc * P:(dc + 1) * P],
                        GTb[:, hc, :],
                        start=(hc == 0), stop=(hc == HC - 1))
                o1 = otp.tile([P, MB], f32, tag="o12")
                nc.scalar.activation(out=o1, in_=ps, func=AF.Identity,
                                     bias=b2c[:, dc:dc + 1])
                o2 = otp.tile([P, MB], f32, tag="o12")
                nc.vector.tensor_tensor(out=o2, in0=o1,
                                        in1=x2Tb[:, dc, :], op=ALU.add)
                for ssc in range(4):
                    tp = tps2.tile([P, P], f32, tag="tp2")
                    nc.tensor.transpose(tp, o2[:, ssc * P:(ssc + 1) * P],
                                        ident)
                    stg = sgp.tile([P, P], f32, tag="stg")
                    if (dc + ssc) % 2 == 0:
                        nc.vector.tensor_copy(stg, tp)
                    else:
                        nc.scalar.copy(stg, tp)
                    r0 = s0 + ssc * P
                    nc.sync.dma_start(
                        out=out_dram[r0:r0 + P, dc * P:(dc + 1) * P],
                        in_=stg)


_CACHED = {}


def _get_nc():
    if "nc" not in _CACHED:
        import concourse.bass as bass
        import concourse.mybir as mybir
        import concourse.tile as tile
        from concourse import bacc
        nc = bacc.Bacc()
        _CACHED["nc"] = build(nc, bass, mybir, tile)
    return _CACHED["nc"]


def _tile_dxd(w):
    """[D, D] -> [out_chunk, d_chunk, d_in, out_in] bf16."""
    return (np.asarray(w, np.float32).astype(ml_dtypes.bfloat16)
            .reshape(DC, P, DC, P).transpose(2, 0, 1, 3).copy())


def prepare_inputs(inputs):
    x = np.asarray(inputs["x"], dtype=np.float32)

    def as_bf16(a):
        return np.asarray(a, dtype=np.float32).astype(ml_dtypes.bfloat16)

    # LN2 fold: w1_eff = w1 * g2 (per input row), w1gs = -sum_d w1_eff[d,h],
    # b1_eff = b1 + w1.T @ ln2_b  (see Phase 4/5 comment)
    w1f = np.asarray(inputs["w1"], np.float32)
    g2 = np.asarray(inputs["ln2_g"], np.float32)
    bln2 = np.asarray(inputs["ln2_b"], np.float32)
    w1_eff = w1f * g2[:, None]
    w1gs_neg = -w1_eff.sum(axis=0)
    b1_eff = np.asarray(inputs["b1"], np.float32) + w1f.T @ bln2
    w1t = (w1_eff.astype(ml_dtypes.bfloat16).reshape(DC, P, HC, P)
           .transpose(2, 0, 1, 3).copy())
    shared = {
        "wq": _tile_dxd(inputs["wq"]), "wk": _tile_dxd(inputs["wk"]),
        "wv": as_bf16(inputs["wv"]), "wo": as_bf16(inputs["wo"]),
        "w1": w1t, "w1gs": w1gs_neg.astype(np.float32),
        "w2": as_bf16(inputs["w2"]),
        "ln1_g": np.asarray(inputs["ln1_g"], np.float32),
        "ln1_b": np.asarray(inputs["ln1_b"], np.float32),
        "ln2_g": g2, "ln2_b": bln2,
        "b1": b1_eff,
        "b2": np.asarray(inputs["b2"], np.float32),
    }
    return [dict(shared, x=np.ascontiguousarray(x[i])) for i in range(N_CORES)]


def kernel(**inputs):
    from concourse.bass_utils import run_bass_kernel_spmd

    nc = _get_nc()
    in_maps = prepare_inputs(inputs)
    res = run_bass_kernel_spmd(nc, in_maps, list(range(N_CORES)))
    out = np.stack([res.results[i]["out"] for i in range(N_CORES)], axis=0)
    return out.astype(np.float32)



# revision 6
# speedup vs baseline: 1.3986x; 1.3986x over previous
"""Trainium2 Bass kernel for a dense transformer block (nn_Block_58377195487260).

Reference (per batch element, fp32):
    h   = LN1(x)*g1 + b1ln
    q,k,v = h@wq, h@wk, h@wv
    s   = q@k^T / sqrt(dk);  a = softmax(s);  y = (a@v)@wo
    x2  = h + y
    mlp = gelu(LN2(x2)*g2 + b2ln @ w1 + b1) @ w2 + b2
    out = x2 + mlp

Sharding: data-parallel over batch. B=8 == 8 NeuronCores; core i computes
batch element i end-to-end (no collectives).

Key optimizations over the straightforward version:
  * Weight folding (host side, fp32): the score matrix is
    s = (h wq)(h wk)^T = h (wq wk^T) h^T, so wqk = wq@wk^T replaces both
    q- and k-projections (one projection instead of two); likewise
    y = a (h wv) wo = a (h (wv wo)), so wvo = wv@wo replaces the
    v-projection + output projection. Attention drops from 34.4 to
    25.8 GFLOP per core and needs only hT + qhT.
  * fp8 (e4m3) DoubleRow matmuls for the whole attention path
    (projections, scores, a@v, softmax partition-sums): 2x tensor-engine
    throughput. Weights are pre-scaled x32 on the host so their sigma~1
    range uses e4m3's mantissa; the 1/32 is folded into the PSUM drain.
    exp() is computed with a -2 bias so ET stays well under e4m3's 240
    max; the bias cancels exactly in the softmax normalization.
    MLP stays bf16: fp8 there pushes max-err past the 2e-2 budget.
  * LN2 is folded into the w1 matmul (w1_eff = w1*g2 pre-scaled;
    mean/var enter via per-column broadcasts), as in the baseline.
  * x2 lives in SBUF as bf16 (bt) for the whole kernel - no DRAM
    round-trip between attention and MLP.
  * All PE transposes run in bf16 (1 cycle/row vs fp32's 2).

Layout is feature-major [d, s] throughout (every bias/gain lands on the
partition axis); the input transpose happens right after LN1 and the
output transpose just before the final DMA.
"""

import numpy as np
import ml_dtypes
from contextlib import ExitStack

P = 128
B, S, D, H = 8, 2048, 1024, 4096
DC = D // P          # 8  d-chunks
HC = H // P          # 32 h-chunks
SC = S // P          # 16 s-chunks
QB = 512             # attention sq-block
NQB = S // QB        # 4
MB = 512             # mlp/ln2 s-block
NMB = S // MB        # 4
EPS = 1e-5
SM_SCALE = 1.0 / 32.0   # 1/sqrt(1024)
WS = 32.0            # host pre-scale on fp8 weights
EXP_BIAS = -2.0      # exp(s*SM + bias); cancels in softmax normalization

N_CORES = 8


def build(nc, bass, mybir, tile):
    f32 = mybir.dt.float32
    bf16 = mybir.dt.bfloat16
    f8 = mybir.dt.float8e4

    x_in = nc.declare_dram_parameter("x", [S, D], f32, isOutput=False)
    wqk_in = nc.declare_dram_parameter("wqk", [D, D], f8, isOutput=False)
    wvo_in = nc.declare_dram_parameter("wvo", [D, D], f8, isOutput=False)
    # w1 arrives pre-scaled by ln2_g and tiled [h_chunk, d_chunk, d_in, h_in];
    # w1gs = -sum_d w1_eff[d,h]; b1 folds in w1.T @ ln2_b
    w1_in = nc.declare_dram_parameter("w1", [HC, DC, P, P], bf16, isOutput=False)
    w1gs_in = nc.declare_dram_parameter("w1gs", [H], f32, isOutput=False)
    w2_in = nc.declare_dram_parameter("w2", [H, D], bf16, isOutput=False)
    ln1g_in = nc.declare_dram_parameter("ln1_g", [D], f32, isOutput=False)
    ln1b_in = nc.declare_dram_parameter("ln1_b", [D], f32, isOutput=False)
    b1_in = nc.declare_dram_parameter("b1", [H], f32, isOutput=False)
    b2_in = nc.declare_dram_parameter("b2", [D], f32, isOutput=False)
    out_dram = nc.declare_dram_parameter("out", [S, D], f32, isOutput=True)

    from concourse.masks import make_identity

    with tile.TileContext(nc) as tc, ExitStack() as top:
        const = top.enter_context(tc.tile_pool(name="const", bufs=1))

        ident = const.tile([P, P], bf16)
        make_identity(nc, ident)
        eps_p = const.tile([P, 1], f32)
        nc.vector.memset(eps_p, EPS)
        eps_1 = const.tile([1, 1], f32)
        nc.vector.memset(eps_1, EPS)
        ones_bf = const.tile([P, 1], bf16)
        nc.vector.memset(ones_bf, 1.0)
        # DoubleRow stationary needs pair-stride %16B == 0, so pad to 16
        ones8_t = const.tile([P, 2, 16], f8)
        nc.vector.memset(ones8_t, 1.0)
        ones8 = ones8_t[:, :, 0:1]
        ones_row = const.tile([1, P], bf16)
        nc.vector.memset(ones_row, 1.0)
        expb_p = const.tile([P, 1], f32)
        nc.vector.memset(expb_p, EXP_BIAS)

        # per-partition views of gains/biases: [P, nchunk], column c = chunk c
        ln1g = const.tile([P, DC], f32)
        ln1b = const.tile([P, DC], f32)
        b1c = const.tile([P, HC], f32)
        b2c = const.tile([P, DC], f32)
        w1gs = const.tile([P, HC], f32)
        for dst, src in ((ln1g, ln1g_in), (ln1b, ln1b_in),
                         (b1c, b1_in), (b2c, b2_in), (w1gs, w1gs_in)):
            nc.sync.dma_start(out=dst, in_=src.rearrange("(c p) -> p c", p=P))

        # x2 in [d, s] layout, bf16, SBUF-resident across attention->MLP
        btp = top.enter_context(tc.tile_pool(name="btp", bufs=1))
        bt = btp.tile([P, DC, S], bf16)

        import os
        for _rep in range(int(os.environ.get("BENCH_REPS", "1"))):
            _build_body(nc, tc, mybir, locals())

    nc.finalize()
    return nc


def _build_body(nc, tc, mybir, env):
    f32 = mybir.dt.float32
    bf16 = mybir.dt.bfloat16
    f8 = mybir.dt.float8e4
    AF = mybir.ActivationFunctionType
    ALU = mybir.AluOpType
    DR = mybir.MatmulPerfMode.DoubleRow
    (x_in, wqk_in, wvo_in, w1_in, w2_in, out_dram,
     ident, eps_p, eps_1, ones_bf, ones8, ones_row, expb_p,
     ln1g, ln1b, b1c, b2c, w1gs, bt) = (
        env["x_in"], env["wqk_in"], env["wvo_in"], env["w1_in"], env["w2_in"],
        env["out_dram"], env["ident"], env["eps_p"], env["eps_1"],
        env["ones_bf"], env["ones8"], env["ones_row"], env["expb_p"],
        env["ln1g"], env["ln1b"], env["b1c"], env["b2c"], env["w1gs"],
        env["bt"])

    with ExitStack() as ph03:
        act = ph03.enter_context(tc.tile_pool(name="act", bufs=1))
        hT = act.tile([P, DC, S], bf16)          # 4 MB, [d, s] (residual)
        h8 = act.tile([P, DC, S], f8)            # 2 MB, fp8 copy for matmuls
        qh8 = act.tile([P, DC, S], f8)           # 2 MB, [d, s] h @ wqk
        VO8 = act.tile([P, SC, D], f8)           # 2 MB, [s, d] h @ wvo
        wkp = ph03.enter_context(tc.tile_pool(name="wkp", bufs=1))
        wqk_sb = wkp.tile([P, DC, D], f8)
        wvo_sb = wkp.tile([P, DC, D], f8)

        # ---- Phase 0-2 fused per 512-col s-block: LN1 + transpose + proj ----
        with ExitStack() as ph:
            xp = ph.enter_context(tc.tile_pool(name="xp", bufs=3))
            hp = ph.enter_context(tc.tile_pool(name="hp", bufs=3))
            st = ph.enter_context(tc.tile_pool(name="st", bufs=4))
            tps = ph.enter_context(
                tc.tile_pool(name="tps", bufs=4, space="PSUM"))
            mps = ph.enter_context(
                tc.tile_pool(name="mps", bufs=4, space="PSUM"))
            wqk_view = wqk_in.rearrange("(c p) n -> p c n", p=P)
            wvo_view = wvo_in.rearrange("(c p) n -> p c n", p=P)
            for sb in range(4):
                for sc in range(4 * sb, 4 * sb + 4):
                    x_t = xp.tile([P, D], f32, tag="x")
                    nc.sync.dma_start(out=x_t, in_=x_in[sc * P:(sc + 1) * P, :])
                    if sc == 0:
                        for g in range(4):
                            nc.sync.dma_start(
                                out=wqk_sb[:, g * 2:(g + 1) * 2, :],
                                in_=wqk_view[:, g * 2:(g + 1) * 2, :])
                            nc.sync.dma_start(
                                out=wvo_sb[:, g * 2:(g + 1) * 2, :],
                                in_=wvo_view[:, g * 2:(g + 1) * 2, :])
                    stats = st.tile([P, 2, 6], f32, tag="stats")
                    nc.vector.bn_stats(out=stats[:, 0, :], in_=x_t[:, 0:512])
                    nc.vector.bn_stats(out=stats[:, 1, :], in_=x_t[:, 512:1024])
                    mv = st.tile([P, 2], f32, tag="mv")
                    nc.vector.bn_aggr(out=mv, in_=stats)
                    std = st.tile([P, 1], f32, tag="std")
                    nc.scalar.activation(out=std, in_=mv[:, 1:2], func=AF.Sqrt,
                                         bias=eps_p)
                    rstd = st.tile([P, 1], f32, tag="rstd")
                    nc.vector.reciprocal(out=rstd, in_=std)
                    h_t = hp.tile([P, D], bf16, tag="h")
                    nc.vector.tensor_scalar(out=h_t, in0=x_t,
                                            scalar1=mv[:, 0:1], scalar2=rstd,
                                            op0=ALU.subtract, op1=ALU.mult)
                    for dc in range(DC):
                        tp = tps.tile([P, P], bf16, tag="tp")
                        nc.tensor.transpose(tp, h_t[:, dc * P:(dc + 1) * P],
                                            ident)
                        nc.vector.tensor_scalar(
                            out=hT[:, dc, sc * P:(sc + 1) * P], in0=tp,
                            scalar1=ln1g[:, dc:dc + 1],
                            scalar2=ln1b[:, dc:dc + 1],
                            op0=ALU.mult, op1=ALU.add)
                        nc.scalar.activation(
                            out=h8[:, dc, sc * P:(sc + 1) * P], in_=tp,
                            func=AF.Identity,
                            bias=ln1b[:, dc:dc + 1],
                            scale=ln1g[:, dc:dc + 1])
                # qh (= h @ wqk) for this s-block, fp8 DoubleRow over dc pairs
                for jc in range(DC):
                    ps = mps.tile([P, QB], f32, tag="ps")
                    for t in range(DC // 2):
                        nc.tensor.matmul(
                            ps, wqk_sb[:, 2 * t:2 * t + 2, jc * P:(jc + 1) * P],
                            h8[:, 2 * t:2 * t + 2, sb * 512:(sb + 1) * 512],
                            start=(t == 0), stop=(t == DC // 2 - 1),
                            perf_mode=DR)
                    o = qh8[:, jc, sb * 512:(sb + 1) * 512]
                    if jc % 2 == 0:
                        nc.vector.tensor_scalar_mul(o, ps, 1.0 / WS)
                    else:
                        nc.scalar.activation(out=o, in_=ps, func=AF.Copy,
                                             scale=1.0 / WS)
                # VO (= h @ wvo) rows for this s-block
                for skc in range(4 * sb, 4 * sb + 4):
                    for db in range(2):
                        ps = mps.tile([P, QB], f32, tag="ps")
                        for t in range(DC // 2):
                            nc.tensor.matmul(
                                ps, h8[:, 2 * t:2 * t + 2, skc * P:(skc + 1) * P],
                                wvo_sb[:, 2 * t:2 * t + 2, db * 512:(db + 1) * 512],
                                start=(t == 0), stop=(t == DC // 2 - 1),
                                perf_mode=DR)
                        o = VO8[:, skc, db * 512:(db + 1) * 512]
                        if (skc + db) % 2 == 0:
                            nc.vector.tensor_scalar_mul(o, ps, 1.0 / WS)
                        else:
                            nc.scalar.activation(out=o, in_=ps, func=AF.Copy,
                                                 scale=1.0 / WS)

        # ------------- Phase 3: attention + residual -> bt -------------
        with ExitStack() as ph:
            etp = ph.enter_context(tc.tile_pool(name="etp", bufs=2))
            rcp = ph.enter_context(tc.tile_pool(name="rcp", bufs=2))
            rbp = ph.enter_context(tc.tile_pool(name="rbp", bufs=2))
            tmp = ph.enter_context(tc.tile_pool(name="tmp", bufs=3))
            sps = ph.enter_context(
                tc.tile_pool(name="sps", bufs=2, space="PSUM"))
            ups = ph.enter_context(
                tc.tile_pool(name="ups", bufs=2, space="PSUM"))
            smps = ph.enter_context(
                tc.tile_pool(name="smps", bufs=2, space="PSUM"))

            for qb in range(NQB):
                q0 = qb * QB
                # ST[sk, sq] = h.T(keys) @ qh(queries); ET = exp(ST/32 - 2)
                ET = etp.tile([P, SC, QB], f8, tag="ET")
                for skc in range(SC):
                    ps = sps.tile([P, QB], f32, tag="st")
                    for t in range(DC // 2):
                        nc.tensor.matmul(
                            ps, h8[:, 2 * t:2 * t + 2, skc * P:(skc + 1) * P],
                            qh8[:, 2 * t:2 * t + 2, q0:q0 + QB],
                            start=(t == 0), stop=(t == DC // 2 - 1),
                            perf_mode=DR)
                    nc.scalar.activation(out=ET[:, skc, :], in_=ps,
                                         func=AF.Exp, scale=SM_SCALE,
                                         bias=expb_p)
                # partition-sum of ET via fp8 ones-matmuls
                sum_ps = smps.tile([1, QB], f32, tag="sm")
                for t in range(SC // 2):
                    nc.tensor.matmul(sum_ps, ones8, ET[:, 2 * t:2 * t + 2, :],
                                     start=(t == 0), stop=(t == SC // 2 - 1),
                                     perf_mode=DR)
                recip = rcp.tile([1, QB], f32, tag="recip")
                nc.vector.reciprocal(out=recip, in_=sum_ps)
                recip_bf = rcp.tile([1, QB], bf16, tag="recipb")
                nc.scalar.copy(recip_bf, recip)
                # broadcast recip over partitions via K=1 bf16 matmul
                rb_ps = smps.tile([P, QB], f32, tag="sm")
                nc.tensor.matmul(rb_ps, ones_row, recip_bf, start=True,
                                 stop=True)
                Rb = rbp.tile([P, QB], f32, tag="Rb")
                nc.vector.tensor_copy(Rb, rb_ps)
                # x2T = hT + (VO.T @ ET) * recip  -> bt (bf16, stays in SBUF)
                for dc in range(DC):
                    ps = ups.tile([P, QB], f32, tag="ps")
                    for t in range(SC // 2):
                        nc.tensor.matmul(
                            ps, VO8[:, 2 * t:2 * t + 2, dc * P:(dc + 1) * P],
                            ET[:, 2 * t:2 * t + 2, :],
                            start=(t == 0), stop=(t == SC // 2 - 1),
                            perf_mode=DR)
                    t1 = tmp.tile([P, QB], f32, tag="t1")
                    nc.vector.tensor_tensor(out=t1, in0=ps, in1=Rb,
                                            op=ALU.mult)
                    nc.vector.tensor_tensor(out=bt[:, dc, q0:q0 + QB],
                                            in0=t1, in1=hT[:, dc, q0:q0 + QB],
                                            op=ALU.add)

    # ------------- Phase 4/5: LN2 + MLP + out -------------
    with ExitStack() as ph:
        w2p = ph.enter_context(tc.tile_pool(name="w2p", bufs=1))
        w1p = ph.enter_context(tc.tile_pool(name="w1p", bufs=6))
        sqp = ph.enter_context(tc.tile_pool(name="sqp", bufs=8))
        gtt = ph.enter_context(tc.tile_pool(name="gtt", bufs=3))
        stp = ph.enter_context(tc.tile_pool(name="stp", bufs=4))
        bcp = ph.enter_context(tc.tile_pool(name="bcp", bufs=4))
        gtp = ph.enter_context(tc.tile_pool(name="gtp", bufs=1))
        otp = ph.enter_context(tc.tile_pool(name="otp", bufs=3))
        o2p = ph.enter_context(tc.tile_pool(name="o2p", bufs=3))
        sgp = ph.enter_context(tc.tile_pool(name="sgp", bufs=6))
        gps = ph.enter_context(tc.tile_pool(name="gps", bufs=2, space="PSUM"))
        mps2 = ph.enter_context(
            tc.tile_pool(name="mps2", bufs=2, space="PSUM"))
        lps = ph.enter_context(tc.tile_pool(name="lps", bufs=2, space="PSUM"))
        tps2 = ph.enter_context(
            tc.tile_pool(name="tps2", bufs=2, space="PSUM"))

        w2_sb = w2p.tile([P, HC, D], bf16)
        w2_view = w2_in.rearrange("(c p) n -> p c n", p=P)
        for g in range(8):
            nc.sync.dma_start(out=w2_sb[:, g * 4:(g + 1) * 4, :],
                              in_=w2_view[:, g * 4:(g + 1) * 4, :])

        for mb in range(NMB):
            s0 = mb * MB
            # LN2 stats: partition sums of x2 and x2^2 (bf16 matmuls on bt)
            sqs = []
            for dc in range(DC):
                sq = sqp.tile([P, MB], bf16, tag="sq")
                nc.scalar.activation(out=sq, in_=bt[:, dc, s0:s0 + MB],
                                     func=AF.Square)
                sqs.append(sq)
            sum_ps = lps.tile([1, MB], f32, tag="lp")
            for dc in range(DC):
                nc.tensor.matmul(sum_ps, ones_bf, bt[:, dc, s0:s0 + MB],
                                 start=(dc == 0), stop=(dc == DC - 1))
            sq_ps = lps.tile([1, MB], f32, tag="lp")
            for dc in range(DC):
                nc.tensor.matmul(sq_ps, ones_bf, sqs[dc],
                                 start=(dc == 0), stop=(dc == DC - 1))
            mu = stp.tile([1, MB], f32, tag="stat")
            nc.scalar.activation(out=mu, in_=sum_ps, func=AF.Copy,
                                 scale=1.0 / D)
            msq = stp.tile([1, MB], f32, tag="stat")
            nc.scalar.activation(out=msq, in_=sq_ps, func=AF.Copy,
                                 scale=1.0 / D)
            var = stp.tile([1, MB], f32, tag="stat")
            nc.vector.tensor_tensor(out=var, in0=mu, in1=mu, op=ALU.mult)
            nc.vector.tensor_tensor(out=var, in0=msq, in1=var,
                                    op=ALU.subtract)
            stdv = stp.tile([1, MB], f32, tag="stat")
            nc.scalar.activation(out=stdv, in_=var, func=AF.Sqrt,
                                 bias=eps_1)
            rstd = stp.tile([1, MB], f32, tag="stat")
            nc.vector.reciprocal(out=rstd, in_=stdv)
            avec = stp.tile([1, MB], f32, tag="stat")
            nc.vector.tensor_tensor(out=avec, in0=mu, in1=rstd, op=ALU.mult)
            rstd_bf = stp.tile([1, MB], bf16, tag="statb")
            nc.scalar.copy(rstd_bf, rstd)
            avec_bf = stp.tile([1, MB], bf16, tag="statb")
            nc.scalar.copy(avec_bf, avec)
            rstd_bc = bcp.tile([P, MB], f32, tag="bc")
            a_bc = bcp.tile([P, MB], f32, tag="bc")
            for vec, bc in ((rstd_bf, rstd_bc), (avec_bf, a_bc)):
                bc_ps = lps.tile([P, MB], f32, tag="lp")
                nc.tensor.matmul(bc_ps, ones_row, vec, start=True,
                                 stop=True)
                nc.vector.tensor_copy(bc, bc_ps)
            # GT = gelu(rstd*(w1g2.T @ x2T) - (mu*rstd)*w1gs + b1_eff):
            # LN2 is folded into the matmul, so PE needn't wait for stats
            GTb = gtp.tile([P, HC, MB], bf16, tag="GTb")
            for hc in range(HC):
                w1t = w1p.tile([P, DC, P], bf16, tag="w1t")
                nc.sync.dma_start(
                    out=w1t, in_=w1_in[hc].rearrange("c p n -> p c n"))
                ps = gps.tile([P, MB], f32, tag="gt")
                for dc in range(DC):
                    nc.tensor.matmul(
                        ps, w1t[:, dc, :], bt[:, dc, s0:s0 + MB],
                        start=(dc == 0), stop=(dc == DC - 1))
                t1 = gtt.tile([P, MB], f32, tag="t1")
                nc.vector.tensor_tensor(out=t1, in0=ps, in1=rstd_bc,
                                        op=ALU.mult)
                t2 = gtt.tile([P, MB], f32, tag="t1")
                nc.vector.scalar_tensor_tensor(
                    out=t2, in0=a_bc, scalar=w1gs[:, hc:hc + 1], in1=t1,
                    op0=ALU.mult, op1=ALU.add)
                nc.scalar.activation(out=GTb[:, hc, :], in_=t2,
                                     func=AF.Gelu,
                                     bias=b1c[:, hc:hc + 1])
            # outT = x2T + w2.T @ GT + b2; transpose to natural layout
            for dc in range(DC):
                ps = mps2.tile([P, MB], f32, tag="mo")
                for hc in range(HC):
                    nc.tensor.matmul(
                        ps, w2_sb[:, hc, dc * P:(dc + 1) * P],
                        GTb[:, hc, :],
                        start=(hc == 0), stop=(hc == HC - 1))
                o1 = otp.tile([P, MB], f32, tag="o1")
                nc.scalar.activation(out=o1, in_=ps, func=AF.Identity,
                                     bias=b2c[:, dc:dc + 1])
                o2 = o2p.tile([P, MB], bf16, tag="o2")
                nc.vector.tensor_tensor(out=o2, in0=o1,
                                        in1=bt[:, dc, s0:s0 + MB], op=ALU.add)
                for ssc in range(4):
                    tp = tps2.tile([P, P], bf16, tag="tp2")
                    nc.tensor.transpose(tp, o2[:, ssc * P:(ssc + 1) * P],
                                        ident)
                    stg = sgp.tile([P, P], f32, tag="stg")
                    if (dc + ssc) % 2 == 0:
                        nc.vector.tensor_copy(stg, tp)
                    else:
                        nc.scalar.copy(stg, tp)
                    r0 = s0 + ssc * P
                    nc.sync.dma_start(
                        out=out_dram[r0:r0 + P, dc * P:(dc + 1) * P],
                        in_=stg)


_CACHED = {}


def _get_nc():
    if "nc" not in _CACHED:
        import concourse.bass as bass
        import concourse.mybir as mybir
        import concourse.tile as tile
        from concourse import bacc
        nc = bacc.Bacc()
        _CACHED["nc"] = build(nc, bass, mybir, tile)
    return _CACHED["nc"]


def prepare_inputs(inputs):
    x = np.asarray(inputs["x"], dtype=np.float32)
    f8 = ml_dtypes.float8_e4m3

    wq = np.asarray(inputs["wq"], np.float32)
    wk = np.asarray(inputs["wk"], np.float32)
    wv = np.asarray(inputs["wv"], np.float32)
    wo = np.asarray(inputs["wo"], np.float32)
    wqk8 = (wq @ wk.T * WS).astype(f8)
    wvo8 = (wv @ wo * WS).astype(f8)

    # LN2 fold: w1_eff = w1 * g2 (per input row), w1gs = -sum_d w1_eff[d,h],
    # b1_eff = b1 + w1.T @ ln2_b
    w1f = np.asarray(inputs["w1"], np.float32)
    g2 = np.asarray(inputs["ln2_g"], np.float32)
    bln2 = np.asarray(inputs["ln2_b"], np.float32)
    w1_eff = w1f * g2[:, None]
    w1gs_neg = -w1_eff.sum(axis=0)
    b1_eff = np.asarray(inputs["b1"], np.float32) + w1f.T @ bln2
    w1t = (w1_eff.astype(ml_dtypes.bfloat16).reshape(DC, P, HC, P)
           .transpose(2, 0, 1, 3).copy())
    shared = {
        "wqk": wqk8, "wvo": wvo8,
        "w1": w1t, "w1gs": w1gs_neg.astype(np.float32),
        "w2": np.asarray(inputs["w2"], np.float32).astype(ml_dtypes.bfloat16),
        "ln1_g": np.asarray(inputs["ln1_g"], np.float32),
        "ln1_b": np.asarray(inputs["ln1_b"], np.float32),
        "b1": b1_eff,
        "b2": np.asarray(inputs["b2"], np.float32),
    }
    return [dict(shared, x=np.ascontiguousarray(x[i])) for i in range(N_CORES)]


def kernel(**inputs):
    from concourse.bass_utils import run_bass_kernel_spmd

    nc = _get_nc()
    in_maps = prepare_inputs(inputs)
    res = run_bass_kernel_spmd(nc, in_maps, list(range(N_CORES)))
    out = np.stack([res.results[i]["out"] for i in range(N_CORES)], axis=0)
    return out.astype(np.float32)
